# revision 2
# baseline (speedup 1.0000x reference)
"""v2: idx-route onehot (bf16 4x DVE), GROUP=16 matmuls, tile_position packing.

Layout notes vs v1:
  - idx = floor(32x) computed exactly: frac = (32x) mod 1 (ts, 2 scalars),
    idx = 32x - frac (stt, bf16 out).  Exact: 32x is a power-of-2 scale,
    mod-1 remainder is exactly representable, difference is an integer <= 31.
  - onehot block j: ts(idx, j, is_equal) in bf16 (4x DVE mode).
  - matmul: GROUP=16 chunks per mm, lhsT=[d|1]x16 [128,32],
    rhs=onehot [128, 32*16=512], out [32, 512] accumulated in PSUM.
    Chunk-groups round-robin over 4 tensor-array col-groups
    (tile_position=(0,32*gc)) for concurrent execution.
  - psum per channel [128, 512]; host extracts diagonal g'==g blocks.
"""

import numpy as np

NB = 32
B, C, H, W = 16, 3, 512, 512
N_CORES = 8
B_PER_CORE = B // N_CORES
ELEMS_PER_CH = B_PER_CORE * H * W
P = 128
F = 512
SUPER_F = 2048
N_SUPER = ELEMS_PER_CH // (P * SUPER_F)
N_SUB = SUPER_F // F
GROUP = 16
NCOLG = 2

_CACHE = {}


def _build(nch=C, n_super=N_SUPER):
    import concourse.bass as bass
    import concourse.tile as tile
    from concourse import bacc, mybir

    nc = bacc.Bacc("TRN2", target_bir_lowering=False, debug=False,
                   num_devices=N_CORES)
    f32 = mybir.dt.float32
    bf16 = mybir.dt.bfloat16

    xin = nc.dram_tensor("xin", [nch, n_super, P, SUPER_F], f32,
                         kind="ExternalInput")
    pin = nc.dram_tensor("pin", [nch, n_super, P, SUPER_F], f32,
                         kind="ExternalInput")
    tin = nc.dram_tensor("tin", [nch, n_super, P, SUPER_F], f32,
                         kind="ExternalInput")
    out = nc.dram_tensor("out", [P, nch * NCOLG * NB * GROUP], f32,
                         kind="ExternalOutput")

    n_groups_per_sub = F // GROUP  # 32
    n_mm_per_cg = n_super * N_SUB * n_groups_per_sub // NCOLG  # per col-group

    with tile.TileContext(nc) as tc:
        with (
            tc.tile_pool(name="inp", bufs=2) as inp,
            tc.tile_pool(name="work", bufs=3) as work,
            tc.tile_pool(name="acc", bufs=1) as accp,
            tc.tile_pool(name="ps", bufs=1, space="PSUM") as ps,
        ):
            psum_all = ps.tile([P, nch, NCOLG, NB * GROUP], f32,
                               tag="psum_all", name="psum_all")

            for c in range(nch):
                mm_cg = [0] * NCOLG
                for sb in range(n_super):
                    xt = inp.tile([P, SUPER_F], f32, tag="x", name="xt")
                    nc.sync.dma_start(out=xt[:], in_=xin[c, sb])
                    pt_in = inp.tile([P, SUPER_F], f32, tag="p", name="pt_in")
                    nc.sync.dma_start(out=pt_in[:], in_=pin[c, sb])
                    tt_in = inp.tile([P, SUPER_F], f32, tag="t", name="tt_in")
                    nc.sync.dma_start(out=tt_in[:], in_=tin[c, sb])

                    for sub in range(N_SUB):
                        xs = xt[:, sub * F:(sub + 1) * F]
                        # Exact floor(32x) via constant-exponent embedding:
                        # w = max(32x, 2^-19) + (32 - 2^-19).  The half-ulp-down
                        # bias makes round-to-nearest place every value in the
                        # correct bin (ties at exact integers round up to the
                        # even mantissa, which is the correct bin); the max()
                        # keeps x < 2^-24 (incl. 0) in bin 0 with exponent 5.
                        # floor(32x) then sits in mantissa bits 18..22 of w.
                        w1 = work.tile([P, F], f32, tag="w1", name="w1")
                        nc.vector.tensor_scalar(
                            out=w1[:], in0=xs, scalar1=float(NB),
                            scalar2=float(2.0 ** -19),
                            op0=mybir.AluOpType.mult,
                            op1=mybir.AluOpType.max)
                        w = work.tile([P, F], f32, tag="w", name="w")
                        nc.vector.tensor_scalar(
                            out=w[:], in0=w1[:],
                            scalar1=float(NB) - float(2.0 ** -19),
                            scalar2=None, op0=mybir.AluOpType.add)
                        idxi = work.tile([P, F], mybir.dt.int32, tag="idxi",
                                         name="idxi")
                        nc.vector.tensor_scalar(
                            out=idxi[:], in0=w[:].bitcast(mybir.dt.int32),
                            scalar1=18, scalar2=NB - 1,
                            op0=mybir.AluOpType.logical_shift_right,
                            op1=mybir.AluOpType.bitwise_and)
                        idxb = work.tile([P, F], bf16, tag="idxb", name="idxb")
                        nc.vector.tensor_copy(out=idxb[:], in_=idxi[:])
                        th = work.tile([P, NB, F], bf16, tag="th", name="th")
                        for j in range(NB):
                            nc.vector.tensor_scalar(
                                out=th[:, j, :], in0=idxb[:],
                                scalar1=float(j), scalar2=None,
                                op0=mybir.AluOpType.is_equal)
                        dn = work.tile([P, F, 2], bf16, tag="dn", name="dn")
                        nc.gpsimd.memset(dn[:], 1.0)
                        nc.vector.tensor_tensor(
                            out=dn[:, :, 0],
                            in0=pt_in[:, sub * F:(sub + 1) * F],
                            in1=tt_in[:, sub * F:(sub + 1) * F],
                            op=mybir.AluOpType.subtract)
                        for gi in range(n_groups_per_sub):
                            f0 = gi * GROUP
                            gc = gi % NCOLG
                            nc.tensor.matmul(
                                psum_all[32 * gc:32 * (gc + 1), c, gc, :],
                                lhsT=dn[:, f0:f0 + GROUP, :],
                                rhs=th[:, :, f0:f0 + GROUP],
                                start=(mm_cg[gc] == 0),
                                stop=(mm_cg[gc] == n_mm_per_cg - 1),
                                tile_position=(0, 32 * gc),
                            )
                            mm_cg[gc] += 1

            res = accp.tile([P, nch * NCOLG * NB * GROUP], f32,
                            name="res")
            nc.gpsimd.memset(res[:], 0.0)
            for c in range(nch):
                for gc in range(NCOLG):
                    nc.vector.tensor_copy(
                        out=res[32 * gc:32 * (gc + 1),
                                (c * NCOLG + gc) * NB * GROUP:
                                (c * NCOLG + gc + 1) * NB * GROUP],
                        in_=psum_all[32 * gc:32 * (gc + 1), c, gc, :])
            nc.sync.dma_start(out=out[:], in_=res[:])

    nc.compile()
    return nc


def _get_nc():
    if "nc" not in _CACHE:
        _CACHE["nc"] = _build()
    return _CACHE["nc"]


def _shard(arr, core):
    a = arr[core * B_PER_CORE:(core + 1) * B_PER_CORE]
    a = np.ascontiguousarray(np.transpose(a, (1, 0, 2, 3)))
    return a.reshape(C, N_SUPER, P, SUPER_F).astype(np.float32, copy=False)


def _decode(raw, nch=C):
    """raw [P, nch*NCOLG*NB*GROUP] -> per-bin (S, Cnt) direct sums."""
    S = np.zeros((nch, NB), np.float64)
    Cnt = np.zeros((nch, NB), np.float64)
    for c in range(nch):
        for gc in range(NCOLG):
            slab = raw[32 * gc:32 * (gc + 1),
                       (c * NCOLG + gc) * NB * GROUP:
                       (c * NCOLG + gc + 1) * NB * GROUP]
            r = slab.reshape(GROUP, 2, NB, GROUP)
            for g in range(GROUP):
                S[c] += r[g, 0, :, g]
                Cnt[c] += r[g, 1, :, g]
    return S, Cnt


def _finalize(S, Cnt):
    diff = np.where(Cnt > 0, np.abs(S) / np.maximum(Cnt, 1.0), 0.0)
    return np.float32(diff.mean())


def kernel(pred, target, input_img):
    from concourse.bass_utils import run_bass_kernel_spmd

    nc = _get_nc()
    in_maps = []
    for core in range(N_CORES):
        in_maps.append({
            "xin": _shard(np.asarray(input_img), core),
            "pin": _shard(np.asarray(pred), core),
            "tin": _shard(np.asarray(target), core),
        })
    res = run_bass_kernel_spmd(nc, in_maps, list(range(N_CORES)))
    S = np.zeros((C, NB), np.float64)
    Cnt = np.zeros((C, NB), np.float64)
    for r in res.results:
        s, cc = _decode(r["out"])
        S += s
        Cnt += cc
    _CACHE["last_SC"] = (S, Cnt)
    return np.asarray(_finalize(S, Cnt), dtype=np.float32)


# revision 3
# speedup vs baseline: 1.0378x; 1.0378x over previous
"""ColorCurveLearningLoss on 8 Trainium2 NeuronCores.

Math: pred_curve - target_curve = sum_bin(pred - target) / count, so the
kernel only needs per-(channel,bin) sums of d = pred - target and counts;
the 8 cores' partials combine by addition on the host, followed by the
division and L1 mean (96 values -- negligible).

Device pipeline per core (data-parallel over batch, 2 images/core):
  - ScalarE: exact floor(32x) prep: w = relu(32x - 2^-19) + 32 puts
    floor(32x) in mantissa bits 18..22 of w (half-ulp-down bias makes
    round-to-nearest land every fp32 value in the correct bin; relu clamps
    x < 2^-24, incl. 0, into bin 0).
  - DVE (VectorE): bitvec extract idx = (w >> 18) & 31 (int32, in place),
    then 32 onehot blocks th[:, j, :] = (idx == j) in bf16 (4x DVE mode --
    this is the throughput wall, at the DVE write-bandwidth floor).
  - ScalarE: idx int32 -> bf16 convert, and the interleaved ones column of
    dn = [d|1] pairs; DVE computes d = pred - target into dn[:, :, 0].
  - TensorE: per 16-chunk group, matmul(lhsT=dn[128, 16, 2] -> M=32,
    rhs=th[128, 32, 16] -> N=512) accumulates [sum_d; count] rows into
    PSUM across all chunks; groups round-robin over 4 tensor-array
    col-groups (tile_position) in 4 separate PSUM banks, drained to SBUF
    per channel.
  - GPSIMD: idle (its elementwise ops contend with DVE for SBUF ports).

d is quantized to bf16 (sums accumulate in fp32 PSUM): final scalar rel
err ~5e-4.  Counts are exact integers (verified elementwise vs numpy).
Measured: ~189 us HW exec per core (VectorE-bound; DMA 83 us, PE 70 us,
ScalarE 60 us).
"""

import numpy as np

NB = 32
B, C, H, W = 16, 3, 512, 512
N_CORES = 8
B_PER_CORE = B // N_CORES
ELEMS_PER_CH = B_PER_CORE * H * W  # 524288
P = 128
F = 1024
SUPER_F = 1024
N_SUPER = ELEMS_PER_CH // (P * SUPER_F)  # 2
N_SUB = SUPER_F // F  # 4
GROUP = 16
NCOLG = 4
N_GP_BLOCKS = 0  # GPSIMD elementwise ops contend for SBUF ports with DVE

_CACHE = {}
_HALF_ULP = float(2.0 ** -19)


def _build(nch=C, n_super=N_SUPER):
    import concourse.bass as bass
    import concourse.tile as tile
    from concourse import bacc, mybir

    nc = bacc.Bacc("TRN2", target_bir_lowering=False, debug=False,
                   num_devices=N_CORES)
    f32 = mybir.dt.float32
    bf16 = mybir.dt.bfloat16
    Relu = mybir.ActivationFunctionType.Relu
    Identity = mybir.ActivationFunctionType.Identity
    Copy = mybir.ActivationFunctionType.Copy

    # activation biases must exist as const APs
    for val in (-_HALF_ULP, float(NB)):
        t = nc.alloc_sbuf_tensor(f"constx-{val}", [128, 1], f32)
        nc.gpsimd.memset(t.ap(), val)
        nc.const_aps.aps[(f32, val)] = t.ap()
    nc.all_engine_barrier()

    xin = nc.dram_tensor("xin", [nch, n_super, P, SUPER_F], f32,
                         kind="ExternalInput")
    pin = nc.dram_tensor("pin", [nch, n_super, P, SUPER_F], f32,
                         kind="ExternalInput")
    tin = nc.dram_tensor("tin", [nch, n_super, P, SUPER_F], f32,
                         kind="ExternalInput")
    out = nc.dram_tensor("out", [P, nch * NB * GROUP], f32,
                         kind="ExternalOutput")

    n_groups_per_sub = F // GROUP  # 32
    n_mm_per_cg = n_super * N_SUB * n_groups_per_sub // NCOLG

    with tile.TileContext(nc) as tc:
        with (
            tc.tile_pool(name="inp", bufs=2) as inp,
            tc.tile_pool(name="work", bufs=2) as work,
            tc.tile_pool(name="acc", bufs=1) as accp,
            tc.tile_pool(name="ps", bufs=1, space="PSUM") as ps,
        ):
            psum_all = ps.tile([P, NCOLG, NB * GROUP], f32,
                               tag="psum_all", name="psum_all")
            res = accp.tile([P, nch * NB * GROUP], f32, name="res")
            nc.vector.memset(res[:], 0.0)

            for c in range(nch):
                mm_cg = [0] * NCOLG
                for sb in range(n_super):
                    xt = inp.tile([P, SUPER_F], f32, tag="x", name="xt")
                    nc.sync.dma_start(out=xt[:], in_=xin[c, sb])
                    pt_in = inp.tile([P, SUPER_F], f32, tag="p", name="pt_in")
                    nc.sync.dma_start(out=pt_in[:], in_=pin[c, sb])
                    tt_in = inp.tile([P, SUPER_F], f32, tag="t", name="tt_in")
                    nc.sync.dma_start(out=tt_in[:], in_=tin[c, sb])

                    for sub in range(N_SUB):
                        xs = xt[:, sub * F:(sub + 1) * F]
                        # ScalarE floor pipeline (see v2 notes):
                        # w = relu(32x - 2^-19) + 32  => exact floor(32x) in
                        # mantissa bits 18..22; x < 2^-24 lands in bin 0.
                        w = work.tile([P, F], f32, tag="w", name="w")
                        nc.scalar.activation(
                            out=w[:], in_=xs, func=Relu,
                            bias=-_HALF_ULP, scale=float(NB))
                        nc.scalar.activation(
                            out=w[:], in_=w[:], func=Identity,
                            bias=float(NB), scale=1.0)
                        wi = w[:].bitcast(mybir.dt.int32)
                        nc.vector.tensor_scalar(
                            out=wi, in0=wi, scalar1=18, scalar2=NB - 1,
                            op0=mybir.AluOpType.logical_shift_right,
                            op1=mybir.AluOpType.bitwise_and)
                        idxb = work.tile([P, F], bf16, tag="idxb", name="idxb")
                        nc.scalar.activation(out=idxb[:], in_=wi, func=Copy)
                        th = work.tile([P, NB, F], bf16, tag="th", name="th")
                        for j in range(NB):
                            nc.vector.tensor_scalar(
                                out=th[:, j, :], in0=idxb[:],
                                scalar1=float(j), scalar2=None,
                                op0=mybir.AluOpType.is_equal)
                        dn = work.tile([P, F, 2], bf16, tag="dn", name="dn")
                        nc.scalar.activation(
                            out=dn[:, :, 1], in_=xs, func=Identity,
                            bias=1.0, scale=0.0)
                        nc.vector.tensor_tensor(
                            out=dn[:, :, 0],
                            in0=pt_in[:, sub * F:(sub + 1) * F],
                            in1=tt_in[:, sub * F:(sub + 1) * F],
                            op=mybir.AluOpType.subtract)
                        for gi in range(n_groups_per_sub):
                            f0 = gi * GROUP
                            gc = gi % NCOLG
                            nc.tensor.matmul(
                                psum_all[32 * gc:32 * (gc + 1), gc, :],
                                lhsT=dn[:, f0:f0 + GROUP, :],
                                rhs=th[:, :, f0:f0 + GROUP],
                                start=(mm_cg[gc] == 0),
                                stop=(mm_cg[gc] == n_mm_per_cg - 1),
                                tile_position=(0, 32 * gc),
                            )
                            mm_cg[gc] += 1

                # drain this channel's psum banks to res rows per col-group
                for gc in range(NCOLG):
                    nc.vector.tensor_copy(
                        out=res[32 * gc:32 * (gc + 1),
                                c * NB * GROUP:(c + 1) * NB * GROUP],
                        in_=psum_all[32 * gc:32 * (gc + 1), gc, :])

            nc.sync.dma_start(out=out[:], in_=res[:])

    nc.compile()
    return nc


def _get_nc():
    if "nc" not in _CACHE:
        _CACHE["nc"] = _build()
    return _CACHE["nc"]


def _shard(arr, core):
    a = arr[core * B_PER_CORE:(core + 1) * B_PER_CORE]
    a = np.ascontiguousarray(np.transpose(a, (1, 0, 2, 3)))
    return a.reshape(C, N_SUPER, P, SUPER_F).astype(np.float32, copy=False)


def _decode(raw, nch=C):
    """raw [P, nch*NB*GROUP]; rows 32gc..32gc+31 hold col-group gc."""
    S = np.zeros((nch, NB), np.float64)
    Cnt = np.zeros((nch, NB), np.float64)
    for c in range(nch):
        slab = raw[:, c * NB * GROUP:(c + 1) * NB * GROUP]
        r = slab.reshape(NCOLG, GROUP, 2, NB, GROUP)
        for g in range(GROUP):
            S[c] += r[:, g, 0, :, g].sum(axis=0)
            Cnt[c] += r[:, g, 1, :, g].sum(axis=0)
    return S, Cnt


def _finalize(S, Cnt):
    diff = np.where(Cnt > 0, np.abs(S) / np.maximum(Cnt, 1.0), 0.0)
    return np.float32(diff.mean())


def kernel(pred, target, input_img):
    from concourse.bass_utils import run_bass_kernel_spmd

    nc = _get_nc()
    in_maps = []
    for core in range(N_CORES):
        in_maps.append({
            "xin": _shard(np.asarray(input_img), core),
            "pin": _shard(np.asarray(pred), core),
            "tin": _shard(np.asarray(target), core),
        })
    res = run_bass_kernel_spmd(nc, in_maps, list(range(N_CORES)))
    S = np.zeros((C, NB), np.float64)
    Cnt = np.zeros((C, NB), np.float64)
    for r in res.results:
        s, cc = _decode(r["out"])
        S += s
        Cnt += cc
    _CACHE["last_SC"] = (S, Cnt)
    return np.asarray(_finalize(S, Cnt), dtype=np.float32)
